# revision 1
# baseline (speedup 1.0000x reference)
"""AlphaWeightedHead Trainium2 kernel: per-sample sigmoid-gated QKV + MHA + proj.

Sharding: data-parallel over batch, 2 samples per core x 8 cores.
All device tensors use a feature-major ("transposed") layout so every matmul
reads its operands in natural orientation (no on-device transposes):

  x^T [c, t]  -> QKV^T [d, t] (Q/K) and V [t, hv]   (contraction over c)
  S^T [tk,tq] = K^T.T @ Q^T per head (contraction over hd, row-packed 2 heads)
  P^T = exp(S^T * scale)  (no max-subtract: |scores| < ~1, data-bounded)
  O^T_aug [65, tq] = [V | ones].T @ P^T  (row 64 = softmax denominator;
      the ones column is folded into the padded V weights host-side)
  Y^T [c_out, t] = pw^T.T @ (O^T / denom)

All matmuls run in bf16 (PSUM accumulates fp32). Host precomputes
sigmoid(alpha[label]), pre-scales V weights/biases, and pads V with the
ones column per head (wv zero-col + bias 1.0 -> V-tile column of ones).
"""

import sys

import numpy as np
import ml_dtypes
from contextlib import ExitStack

try:
    import concourse.bass as _probe  # noqa: F401
except ModuleNotFoundError:
    sys.path.insert(0, "/opt/trn_rl_repo")

import concourse.bass as bass
import concourse.bacc as bacc
import concourse.tile as tile
from concourse import mybir
from concourse.bass_utils import run_bass_kernel_spmd

B, NT, C, H, CLS = 16, 1024, 768, 12, 1000
HD = 64
NCORES = 8
SPC = B // NCORES          # samples per core = 2
T = SPC * NT               # tokens per core = 2048
NPAIR = H // 2             # 6 head pairs
CH = C // 128              # 6 contraction chunks
CP = H * (HD + 1)          # padded V width = 780 (65 per head)
SCALE = HD ** -0.5

F32 = mybir.dt.float32
BF16 = mybir.dt.bfloat16
ADD = mybir.AluOpType.add
MULT = mybir.AluOpType.mult
EXP = mybir.ActivationFunctionType.Exp


def build(debug=False, phases="all"):
    nc = bacc.Bacc("TRN2")
    xt = nc.declare_dram_parameter("xt", [C, T], BF16, isOutput=False)
    wqk = nc.declare_dram_parameter("wqk", [C, 2 * C], BF16, isOutput=False)
    wv = nc.declare_dram_parameter("wv", [SPC, C, CP], BF16, isOutput=False)
    sigbq = nc.declare_dram_parameter("sigbq", [128, 2 * SPC * 12], F32,
                                      isOutput=False)
    bvs = nc.declare_dram_parameter("bvs", [SPC, CP], BF16, isOutput=False)
    pw = nc.declare_dram_parameter("pw", [C, C], BF16, isOutput=False)
    pb = nc.declare_dram_parameter("pb", [128, CH], F32, isOutput=False)
    out = nc.declare_dram_parameter("out", [C, T], F32, isOutput=True)
    if debug:
        dbg_v = nc.declare_dram_parameter("dbg_v", [128, 8 * CP], F32, isOutput=True)
        dbg_qk = nc.declare_dram_parameter("dbg_qk", [128, 2 * T], F32, isOutput=True)
        dbg_pt = nc.declare_dram_parameter("dbg_pt", [128, 2 * 8 * 512], F32, isOutput=True)
        dbg_st = nc.declare_dram_parameter("dbg_st", [128, 2 * 512], F32, isOutput=True)
        dbg_ob = nc.declare_dram_parameter("dbg_ob", [128, CH * T], F32, isOutput=True)

    with tile.TileContext(nc) as tc, ExitStack() as ctx:
        cpool = ctx.enter_context(tc.tile_pool(name="const", bufs=1))
        wvp = ctx.enter_context(tc.tile_pool(name="wvp", bufs=1))
        wqkp = ctx.enter_context(tc.tile_pool(name="wqkp", bufs=2))
        qkp = ctx.enter_context(tc.tile_pool(name="qkp", bufs=2))
        ptp = ctx.enter_context(tc.tile_pool(name="ptp", bufs=4))
        stgp = ctx.enter_context(tc.tile_pool(name="stgp", bufs=4))
        rqp = ctx.enter_context(tc.tile_pool(name="rqp", bufs=4))
        yp = ctx.enter_context(tc.tile_pool(name="yp", bufs=3))
        dramp = ctx.enter_context(
            tc.tile_pool(name="dramp", bufs=4, space=bass.MemorySpace.DRAM))
        mmps = ctx.enter_context(
            tc.tile_pool(name="mmps", bufs=2, space=bass.MemorySpace.PSUM))
        stps = ctx.enter_context(
            tc.tile_pool(name="stps", bufs=2, space=bass.MemorySpace.PSUM))
        pvps = ctx.enter_context(
            tc.tile_pool(name="pvps", bufs=2, space=bass.MemorySpace.PSUM))

        # ---- resident tensors
        xt_sb = cpool.tile([128, CH, T], BF16)
        for c in range(CH):
            nc.sync.dma_start(xt_sb[:, c, :], xt[c * 128:(c + 1) * 128, :])
        pw_sb = cpool.tile([128, CH, C], BF16)
        nc.sync.dma_start(pw_sb[:], pw.rearrange("(c p) n -> p c n", p=128))
        sigbq_sb = cpool.tile([128, 2 * SPC * 12], F32)
        nc.sync.dma_start(sigbq_sb[:], sigbq[:])
        pb_sb = cpool.tile([128, CH], F32)
        nc.sync.dma_start(pb_sb[:], pb[:])
        bvs_sb = cpool.tile([128, SPC, CP], BF16)
        for s in range(SPC):
            nc.sync.dma_start(
                bvs_sb[:, s:s + 1, :], bvs[s:s + 1, :].partition_broadcast(128))
        ob = cpool.tile([128, CH, T], BF16)
        ones_t = cpool.tile([128, 64], BF16)
        nc.vector.memset(ones_t[64:65, :], 1.0)
        vbs = [cpool.tile([128, 8, CP], BF16, tag=f"vb{s}", name=f"vb{s}")
               for s in range(SPC)]

        # ---- V phase: V_pad[t, 780] = x_s @ wv_pad + bvs_pad  (per sample)
        # wv_pad has a zero column per head; bvs_pad carries 1.0 there, so
        # the padded column becomes the all-ones denominator column.
        for s in range(SPC):
            wv_sb = wvp.tile([128, CH, CP], BF16)
            nc.sync.dma_start(wv_sb[:], wv[s].rearrange("(c p) n -> p c n", p=128))
            for tt in range(8):
                for hvt in range(2):
                    h0 = hvt * 512
                    hvn = 512 if hvt == 0 else CP - 512
                    ps = mmps.tile([128, 512], F32, tag="mm")
                    for c in range(CH):
                        nc.tensor.matmul(
                            ps[:, :hvn],
                            xt_sb[:, c, s * NT + tt * 128: s * NT + (tt + 1) * 128],
                            wv_sb[:, c, h0: h0 + hvn],
                            start=(c == 0), stop=(c == CH - 1),
                        )
                    nc.vector.tensor_add(
                        vbs[s][:, tt, h0:h0 + hvn],
                        ps[:, :hvn],
                        bvs_sb[:, s, h0:h0 + hvn])
                    if debug and s == 0:
                        dv = yp.tile([128, 512], F32, tag="dbgv", name="dv")
                        nc.vector.tensor_copy(dv[:, :hvn], vbs[s][:, tt, h0:h0 + hvn])
                        nc.sync.dma_start(
                            dbg_v[:, tt * CP + h0: tt * CP + h0 + hvn], dv[:, :hvn])

        # ---- PV + epilogue, one iteration behind S^T/exp (keeps ACT fed)
        pending = []

        def emit_pv(item):
            p, s, tq, pt = item
            for hh in range(2):
                h = 2 * p + hh
                pv = pvps.tile([128, 512], F32, tag="pv", name="pv")
                for chk in range(8):
                    nc.tensor.matmul(
                        pv[0:65, :],
                        vbs[s][:, chk, h * 65: h * 65 + 65],
                        pt[hh][:, chk, :],
                        start=(chk == 0), stop=(chk == 7),
                    )
                stg = stgp.tile([128, 512], BF16, tag="stg", name="stg")
                nc.vector.tensor_copy(stg[0:65, :], pv[0:65, :])
                with nc.allow_low_precision(reason="softmax denom bf16"):
                    nc.vector.reciprocal(stg[64:65, :], stg[64:65, :])
                rb = pvps.tile([128, 512], F32, tag="pv", name="rb")
                nc.tensor.matmul(
                    rb[0:64, :],
                    ones_t[64:65, :],
                    stg[64:65, :],
                    start=True, stop=True,
                    tile_position=(64, 0),
                )
                if hh == 0:
                    nc.vector.scalar_tensor_tensor(
                        ob[0:64, p, s * NT + tq * 512: s * NT + (tq + 1) * 512],
                        stg[0:64, :], 0.0, rb[0:64, :],
                        mybir.AluOpType.bypass, MULT)
                else:
                    stn = stgp.tile([64, 512], BF16, tag="stn", name="stn")
                    nc.vector.scalar_tensor_tensor(
                        stn[:], stg[0:64, :], 0.0, rb[0:64, :],
                        mybir.AluOpType.bypass, MULT)
                    nc.gpsimd.dma_start(
                        ob[64:128, p,
                           s * NT + tq * 512: s * NT + (tq + 1) * 512],
                        stn[:])

        # ---- head-pair loop: QKV(Q,K) -> S^T -> exp -> PV -> normalize
        for p in range(NPAIR if phases != "v" else 0):
            wqk_t = wqkp.tile([128, CH, 256], BF16)
            nc.sync.dma_start(
                wqk_t[:, :, 0:128],
                wqk[:, p * 128:(p + 1) * 128].rearrange("(c p) n -> p c n", p=128))
            nc.sync.dma_start(
                wqk_t[:, :, 128:256],
                wqk[:, C + p * 128: C + (p + 1) * 128].rearrange(
                    "(c p) n -> p c n", p=128))
            qk_t = qkp.tile([128, 2, T], BF16)
            for qk in range(2):
                for n in range(4):
                    ps = mmps.tile([128, 512], F32, tag="mm")
                    for c in range(CH):
                        nc.tensor.matmul(
                            ps[:],
                            wqk_t[:, c, qk * 128:(qk + 1) * 128],
                            xt_sb[:, c, n * 512:(n + 1) * 512],
                            start=(c == 0), stop=(c == CH - 1),
                        )
                    s = n // 2
                    j = s * 12 + qk * 6 + p
                    nc.vector.tensor_scalar(
                        qk_t[:, qk, n * 512:(n + 1) * 512], ps[:],
                        sigbq_sb[:, j:j + 1], sigbq_sb[:, 24 + j:24 + j + 1],
                        MULT, ADD)
                    if debug and p == 0:
                        dq = yp.tile([128, 512], F32, tag="dbgq", name="dq")
                        nc.vector.tensor_copy(
                            dq[:], qk_t[:, qk, n * 512:(n + 1) * 512])
                        nc.sync.dma_start(
                            dbg_qk[:, qk * T + n * 512: qk * T + (n + 1) * 512],
                            dq[:])

            for s in range(SPC if phases not in ("v", "vqk") else 0):
                for tq in range(2):
                    pt = [ptp.tile([128, 8, 512], BF16, tag="pt", name=f"pt{_h}")
                          for _h in range(2)]
                    for tk2 in range(4):
                        st2 = [stps.tile([128, 2, 512], F32, tag="st",
                                         name=f"st{_h}") for _h in range(2)]
                        for sub in range(2):
                            tk = 2 * tk2 + sub
                            for hh in range(2):
                                lo = hh * 64
                                nc.tensor.matmul(
                                    st2[hh][:, sub, :],
                                    qk_t[lo:lo + 64, 1,
                                         s * NT + tk * 128: s * NT + (tk + 1) * 128],
                                    qk_t[lo:lo + 64, 0,
                                         s * NT + tq * 512: s * NT + (tq + 1) * 512],
                                    start=True, stop=True,
                                    tile_position=(lo, 0),
                                )
                        for hh in range(2):
                            nc.scalar.activation(
                                pt[hh][:, 2 * tk2:2 * tk2 + 2, :],
                                st2[hh][:], EXP, scale=SCALE)
                    pending.append((p, s, tq, pt))
                    if len(pending) > 1:
                        emit_pv(pending.pop(0))
        while pending:
            emit_pv(pending.pop(0))

        if debug and phases == "all":
            for c in range(CH):
                for n in range(4):
                    do = yp.tile([128, 512], F32, tag="dbgo", name="do")
                    nc.vector.tensor_copy(do[:], ob[:, c, n * 512:(n + 1) * 512])
                    nc.sync.dma_start(
                        dbg_ob[:, c * T + n * 512: c * T + (n + 1) * 512], do[:])
        # ---- proj: Y^T = pw^T.T @ O^T + pb
        for m in range(CH if phases == "all" else 0):
            for n in range(4):
                ps = mmps.tile([128, 512], F32, tag="mm")
                for c in range(CH):
                    nc.tensor.matmul(
                        ps[:],
                        pw_sb[:, c, m * 128:(m + 1) * 128],
                        ob[:, c, n * 512:(n + 1) * 512],
                        start=(c == 0), stop=(c == CH - 1),
                    )
                y_t = yp.tile([128, 512], F32)
                nc.vector.tensor_scalar(y_t[:], ps[:], pb_sb[:, m:m + 1], None, ADD)
                nc.gpsimd.dma_start(
                    out[m * 128:(m + 1) * 128, n * 512:(n + 1) * 512], y_t[:])
    nc.compile()
    return nc


def make_in_maps(x, label, alpha, qkv_w, qkv_b, proj_w, proj_b):
    x = np.asarray(x, np.float32)
    label = np.asarray(label)
    alpha = np.asarray(alpha, np.float32)
    qkv_w = np.asarray(qkv_w, np.float32)
    qkv_b = np.asarray(qkv_b, np.float32)
    proj_w = np.asarray(proj_w, np.float32)
    proj_b = np.asarray(proj_b, np.float32)

    sig = 1.0 / (1.0 + np.exp(-alpha[label]))          # (B, 3C) f32
    wqkT = np.ascontiguousarray(qkv_w[:2 * C].T).astype(ml_dtypes.bfloat16)
    wvT = np.ascontiguousarray(qkv_w[2 * C:].T)         # (C, C) f32
    pw_bf = np.ascontiguousarray(proj_w.T).astype(ml_dtypes.bfloat16)
    pb_arr = np.ascontiguousarray(proj_b.reshape(CH, 128).T)

    in_maps = []
    for i in range(NCORES):
        sl = slice(SPC * i, SPC * (i + 1))
        xs = x[sl]                                      # (2, NT, C)
        xt = np.ascontiguousarray(
            xs.transpose(2, 0, 1).reshape(C, T)).astype(ml_dtypes.bfloat16)
        sig_i = sig[sl]                                 # (2, 3C)
        sqk = sig_i[:, :2 * C]                          # (2, 2C)
        sq = sqk.reshape(SPC, 12, 128).transpose(2, 0, 1).reshape(128, SPC * 12)
        bq = ((qkv_b[None, :2 * C] * sqk).reshape(SPC, 12, 128)
              .transpose(2, 0, 1).reshape(128, SPC * 12))
        sigbq_i = np.ascontiguousarray(np.concatenate([sq, bq], axis=1))
        sigv = sig_i[:, 2 * C:]                         # (2, C)
        wv_sc = wvT[None, :, :] * sigv[:, None, :]      # (2, C, C)
        wv_pad = np.zeros((SPC, C, CP), np.float32)
        bvs_pad = np.zeros((SPC, CP), np.float32)
        for h in range(H):
            wv_pad[:, :, h * 65:h * 65 + 64] = wv_sc[:, :, h * 64:(h + 1) * 64]
            bvs_pad[:, h * 65:h * 65 + 64] = (
                qkv_b[None, 2 * C + h * 64: 2 * C + (h + 1) * 64]
                * sigv[:, h * 64:(h + 1) * 64])
            bvs_pad[:, h * 65 + 64] = 1.0
        in_maps.append({
            "xt": xt, "wqk": wqkT,
            "wv": np.ascontiguousarray(wv_pad).astype(ml_dtypes.bfloat16),
            "sigbq": sigbq_i,
            "bvs": np.ascontiguousarray(bvs_pad).astype(ml_dtypes.bfloat16),
            "pw": pw_bf, "pb": pb_arr,
        })
    return in_maps


_NC = None
LAST_RESULT = None


def kernel(x, label, alpha, qkv_w, qkv_b, proj_w, proj_b):
    global _NC, LAST_RESULT
    if _NC is None:
        _NC = build()
    in_maps = make_in_maps(x, label, alpha, qkv_w, qkv_b, proj_w, proj_b)
    res = run_bass_kernel_spmd(_NC, in_maps, core_ids=list(range(NCORES)))
    LAST_RESULT = res
    outs = []
    for i in range(NCORES):
        y = np.asarray(res.results[i]["out"])           # (C, T)
        outs.append(y.reshape(C, SPC, NT).transpose(1, 2, 0))
    return np.ascontiguousarray(np.concatenate(outs, axis=0), dtype=np.float32)

